# revision 1
# baseline (speedup 1.0000x reference)
"""Trainium2 Bass kernel for DoubleHeadRNN (two independent GRUs over the same input).

Problem: x [64, 1024, 512]; two Keras-style GRUCells (reset_after=True) with
H=1024, T=1024 steps; returns (h_last_head0, h_last_head1).

Strategy (v2): one head per core (cores 0/1 produce the two heads; the SPMD
program is identical on all 8 cores). Per step the fused projection
g = [x_t; h] @ [W; U] runs as PE matmuls with h kept transposed (regenerated
each step by PE transposes). The candidate gate needs xh and hh separately
(h_cand = tanh(xh + r*hh)), so PSUM keeps [zneg | r | xh | hh] regions.
z columns are negated on host so one sigmoid yields zneg = 1-z directly:
    h_new = h + zneg * (cand - h)

Performance structure: the PE array is column-split into two concurrent
32-col-group tiles (tile_position=(0,0) and (0,64)); tile `t` computes a
*different* 256-wide slice of the H columns, so no partial-combination is
needed and every ACT/DVE gate op runs on all 128 partitions
(parts 0-63 = batch for tile0's slice, parts 64-127 = batch for tile1's).
H is processed in two halves per step (psum [128, 1024] = 2 banks, bufs=2
so halves and steps pipeline). All column/row permutations that this
storage order implies are folded into the host-side weight layout.

Storage order: H-natural index n = 512*h + 256*t + w lives at
h_cur[64*t + b, 256*h + w] (h = half, t = col-tile).
"""

import os
import numpy as np
from contextlib import ExitStack

B, T, D, H = 64, 1024, 512, 1024
KC = (D + H) // 128  # 12 K-chunks of the fused contraction
NCORES = 8

_cache = {}


def _build(n_steps, bf16=False):
    import concourse.bass as bass
    import concourse.tile as tile
    from concourse import bacc, mybir

    f32 = mybir.dt.float32
    r32 = mybir.dt.float32r
    # float32r: same 4-byte storage, PE streams 1 cycle/row vs fp32's 4.
    # All matmul-feeding tensors (xt, wu, hT) are declared float32r; the
    # hT copy from psum performs the required fp32r rounding.
    mdt = mybir.dt.bfloat16 if bf16 else r32
    AF = mybir.ActivationFunctionType

    def rc(ap):
        # PE runs fp32 matmuls at 4 cycles/row but float32r (same 4-byte
        # storage, reduced-precision multiply) at 1 cycle/row for N>=256.
        return ap.bitcast(r32)

    nc = bacc.Bacc(
        "TRN2", target_bir_lowering=False, debug=False, num_devices=NCORES
    )
    xt_d = nc.dram_tensor("xt", [n_steps * 128, 256], mdt, kind="ExternalInput").ap()
    wu_d = nc.dram_tensor("wu", [KC * 128, 3072], mdt, kind="ExternalInput").ap()
    id_d = nc.dram_tensor("ident", [128, 64], f32, kind="ExternalInput").ap()
    out_d = nc.dram_tensor("out", [64, 1024], f32, kind="ExternalOutput").ap()

    with tile.TileContext(nc) as tc, ExitStack() as ctx:
        const = ctx.enter_context(tc.tile_pool(name="const", bufs=1))
        state = ctx.enter_context(tc.tile_pool(name="state", bufs=1))
        xpool = ctx.enter_context(tc.tile_pool(name="xin", bufs=4))
        gates = ctx.enter_context(tc.tile_pool(name="gates", bufs=3))
        ppool = ctx.enter_context(tc.tile_pool(name="psum", bufs=2, space="PSUM"))
        xpsum = ctx.enter_context(tc.tile_pool(name="psumX", bufs=1, space="PSUM"))
        tpool = ctx.enter_context(tc.tile_pool(name="psumT", bufs=1, space="PSUM"))

        # --- persistent SBUF ---
        wu_s = const.tile([128, KC * 3072], mdt, tag="wu")
        for c in range(KC):
            nc.sync.dma_start(
                wu_s[:, c * 3072 : (c + 1) * 3072],
                wu_d[c * 128 : (c + 1) * 128, :],
            )
        ident = const.tile([128, 64], f32, tag="ident")
        nc.sync.dma_start(ident[:], id_d[:])

        # h state, parity pairs ([128, 512] storage order, see module docstring)
        h_cur = [state.tile([64, 1024], f32, tag=f"hcur{p}", name=f"hcur{p}") for p in range(2)]
        hT = [state.tile([128, 512], mdt, tag=f"hT{p}", name=f"hT{p}") for p in range(2)]
        nc.vector.memset(h_cur[0][:], 0.0)
        nc.vector.memset(hT[0][:].bitcast(f32), 0.0)

        def step(iv, p):
            """One GRU step reading state parity p, writing parity 1-p."""
            h_in, hT_in = h_cur[p], hT[p]
            h_out, hT_out = h_cur[1 - p], hT[1 - p]

            xt_t = xpool.tile([128, 256], mdt, tag="xt")
            nc.sync.dma_start(xt_t[:], xt_d[bass.ds(iv * 128, 128), :])

            h_new = h_out

            for hf in range(2):  # halves of H
                # psum ps [64, 1536]: [zneg 512 | r 512 | hh 512]; xh separate
                ps = ppool.tile([64, 1536], f32, tag="ps")
                xh = xpsum.tile([64, 512], f32, tag="xh")
                for c in range(KC):
                    lhsT = (
                        xt_t[:, c * 64 : (c + 1) * 64]
                        if c < 4
                        else hT_in[:, (c - 4) * 64 : (c - 3) * 64]
                    )
                    wb = c * 3072 + hf * 512
                    nc.tensor.matmul(
                        ps[:, 0:512], lhsT, wu_s[:, wb : wb + 512],
                        start=(c == 0), stop=(c == KC - 1), skip_group_check=True,
                    )
                    nc.tensor.matmul(
                        ps[:, 512:1024], lhsT, wu_s[:, wb + 1024 : wb + 1536],
                        start=(c == 0), stop=(c == KC - 1), skip_group_check=True,
                    )
                    if c < 4:
                        nc.tensor.matmul(
                            xh[:, 0:512], lhsT,
                            wu_s[:, wb + 2048 : wb + 2560],
                            start=(c == 0), stop=(c == 3), skip_group_check=True,
                        )
                    else:
                        nc.tensor.matmul(
                            ps[:, 1024:1536], lhsT,
                            wu_s[:, wb + 2048 : wb + 2560],
                            start=(c == 4), stop=(c == KC - 1), skip_group_check=True,
                        )

                zr = gates.tile([64, 1024], f32, tag="zr")
                nc.scalar.activation(zr[:], ps[:, 0:1024], AF.Sigmoid)
                t1 = gates.tile([64, 512], f32, tag="t1")
                nc.vector.tensor_mul(t1[:], zr[:, 512:1024], ps[:, 1024:1536])
                t2 = gates.tile([64, 512], f32, tag="t2")
                nc.vector.tensor_add(t2[:], t1[:], xh[:])
                cand = gates.tile([64, 512], f32, tag="cand")
                nc.scalar.activation(cand[:], t2[:], AF.Tanh)
                hs = h_in[:, hf * 512 : (hf + 1) * 512]
                d = gates.tile([64, 512], f32, tag="d")
                nc.vector.tensor_sub(d[:], cand[:], hs)
                e = gates.tile([64, 512], f32, tag="e")
                nc.vector.tensor_mul(e[:], zr[:, 0:512], d[:])
                nc.vector.tensor_add(h_new[:, hf * 512 : (hf + 1) * 512], hs, e[:])

            # update state: transpose h_new (== h_out) -> hT_out
            pt = tpool.tile([128, 512], f32, tag="pt")
            for k in range(8):
                nc.tensor.transpose(
                    pt[:, k * 64 : (k + 1) * 64],
                    h_new[:, k * 128 : (k + 1) * 128],
                    ident[0:64, :],
                )
            # split copy: chunks 0-3 land early so next step's first h-MMs
            # need not wait for half1's transposes
            nc.vector.tensor_copy(hT_out[:, 0:256], pt[:, 0:256])
            nc.vector.tensor_copy(hT_out[:, 256:512], pt[:, 256:512])

        with tc.For_i(0, n_steps, 4, hint_engines=(mybir.EngineType.PE,), staggered_reset=True) as i:
            step(i, 0)
            step(i + 1, 1)
            step(i + 2, 0)
            step(i + 3, 1)

        nc.sync.dma_start(out_d[:], h_cur[0][:])

    nc.compile()
    return nc


def _col_perm():
    """Natural column order: [zneg 1024 | r 1024 | hc 1024]."""
    return np.arange(3 * H, dtype=np.int64)


def _row_perm_u():
    """Natural U-row order (h stored unpermuted)."""
    return np.arange(H, dtype=np.int64)


_CPERM = _col_perm()
_RPERM = _row_perm_u()


def _host_prep(x, W, U, bf16=False):
    """Build xt / wu host-side arrays for one head."""
    n_steps = x.shape[1]
    xt = (
        x.transpose(1, 2, 0)                      # [T, D, B]
        .reshape(n_steps, 4, 128, B)              # [T, c, p, b]
        .transpose(0, 2, 1, 3)                    # [T, p, c, b]
        .reshape(n_steps * 128, 256)
        .astype(np.float32)
    )
    Wp = np.asarray(W, np.float32)[:, _CPERM]
    Up = np.asarray(U, np.float32)[_RPERM][:, _CPERM]
    wu = np.concatenate([Wp, Up], axis=0).copy()  # [1536, 3072]
    # negate z columns
    wu[:, 0:H] *= -1.0
    if bf16:
        import ml_dtypes
        xt = xt.astype(ml_dtypes.bfloat16)
        wu = wu.astype(ml_dtypes.bfloat16)
    return np.ascontiguousarray(xt), np.ascontiguousarray(wu)


def _unpermute_h(res):
    """h is stored in natural order now."""
    return np.asarray(res, np.float32)


def _run_spmd(nc, in_maps, n_timed=0):
    """Execute on the 8 axon cores via PJRT shard_map; keeps the jitted
    callable + device inputs resident so timed runs measure execution."""
    import time
    import jax
    from jax.sharding import Mesh, PartitionSpec
    from jax.experimental.shard_map import shard_map
    from concourse import bass2jax, mybir

    bass2jax.install_neuronx_cc_hook()
    n_cores = len(in_maps)

    in_names, out_names, out_avals = [], [], []
    partition_name = nc.partition_id_tensor.name if nc.partition_id_tensor else None
    for alloc in nc.m.functions[0].allocations:
        if not isinstance(alloc, mybir.MemoryLocationSet):
            continue
        name = alloc.memorylocations[0].name
        if alloc.kind == "ExternalInput":
            if name != partition_name:
                in_names.append(name)
        elif alloc.kind == "ExternalOutput":
            shape = tuple(alloc.tensor_shape)
            dtype = mybir.dt.np(alloc.dtype)
            out_avals.append(jax.core.ShapedArray(shape, dtype))
            out_names.append(name)
    n_params = len(in_names)
    n_outs = len(out_names)
    all_in = in_names + out_names
    if partition_name is not None:
        all_in.append(partition_name)

    def _body(*args):
        operands = list(args)
        if partition_name is not None:
            operands.append(bass2jax.partition_id_tensor())
        outs = bass2jax._bass_exec_p.bind(
            *operands,
            out_avals=tuple(out_avals),
            in_names=tuple(all_in),
            out_names=tuple(out_names),
            lowering_input_output_aliases=(),
            sim_require_finite=True,
            sim_require_nnan=True,
            nc=nc,
        )
        return tuple(outs)

    devices = jax.devices()[:n_cores]
    mesh = Mesh(np.asarray(devices), ("core",))
    in_specs = (PartitionSpec("core"),) * (n_params + n_outs)
    out_specs = (PartitionSpec("core"),) * n_outs
    sharded = jax.jit(
        shard_map(_body, mesh=mesh, in_specs=in_specs, out_specs=out_specs,
                  check_rep=False),
        keep_unused=True,
    )
    sharding = jax.sharding.NamedSharding(mesh, PartitionSpec("core"))

    def _stage(per_core_arrays):
        shards = []
        for c, arr in enumerate(per_core_arrays):
            sh = jax.device_put(np.asarray(arr), devices[c])
            sh.block_until_ready()
            shards.append(sh)
        a0 = np.asarray(per_core_arrays[0])
        gshape = (n_cores * a0.shape[0], *a0.shape[1:])
        return jax.make_array_from_single_device_arrays(gshape, sharding, shards)

    dev_in = [_stage([in_maps[c][nm] for c in range(n_cores)]) for nm in in_names]
    dev_zero = [
        _stage([np.zeros(av.shape, av.dtype) for _ in range(n_cores)])
        for av in out_avals
    ]
    for a in dev_in + dev_zero:
        a.block_until_ready()

    out_arrs = sharded(*dev_in, *dev_zero)
    jax.block_until_ready(out_arrs)

    best = None
    for _ in range(n_timed):
        t0 = time.perf_counter_ns()
        out_arrs = sharded(*dev_in, *dev_zero)
        jax.block_until_ready(out_arrs)
        dt = time.perf_counter_ns() - t0
        best = dt if best is None else min(best, dt)

    results = [
        {
            nm: np.asarray(out_arrs[i]).reshape(n_cores, *out_avals[i].shape)[c]
            for i, nm in enumerate(out_names)
        }
        for c in range(n_cores)
    ]
    return results, best


def _make_ident():
    id2 = np.zeros((128, 64), np.float32)
    for p in range(128):
        id2[p, p % 64] = 1.0
    return id2


def kernel(x, W0, U0, bi0, br0, W1, U1, bi1, br1):
    x = np.asarray(x, dtype=np.float32)
    assert all(
        not np.any(np.asarray(b)) for b in (bi0, br0, bi1, br1)
    ), "nonzero biases not supported by this kernel build"

    bf16 = bool(int(os.environ.get("GRU_BF16", "0")))
    n_steps = x.shape[1]
    key = (n_steps, bf16)
    if key not in _cache:
        _cache[key] = _build(n_steps, bf16=bf16)
    nc = _cache[key]

    xt, wu0 = _host_prep(x, np.asarray(W0), np.asarray(U0), bf16=bf16)
    _, wu1 = _host_prep(x[:, :1], np.asarray(W1), np.asarray(U1), bf16=bf16)
    ident = _make_ident()

    maps = []
    for core in range(NCORES):
        wu = wu0 if core % 2 == 0 else wu1
        maps.append({"xt": xt, "wu": wu, "ident": ident})

    n_timed = int(os.environ.get("GRU_TIMED_RUNS", "0"))
    results, best_ns = _run_spmd(nc, maps, n_timed=n_timed)
    kernel.last_exec_ns = best_ns
    out0 = _unpermute_h(results[0]["out"])
    out1 = _unpermute_h(results[1]["out"])
    return out0, out1


kernel.last_exec_ns = None



# revision 10
# speedup vs baseline: 1.0785x; 1.0785x over previous
"""Trainium2 Bass kernel for DoubleHeadRNN (two independent GRUs over the same input).

Problem: x [64, 1024, 512]; two Keras-style GRUCells (reset_after=True) with
H=1024, T=1024 steps; returns (h_last_head0, h_last_head1).

Strategy (v5): one head per core (cores 0/1 produce the two heads; the SPMD
program is identical on all 8 cores).

The input projection gx = x_t @ W is NOT computed inside the recurrence.
Instead it is precomputed by the same core in 2-timestep-stacked form: the
stationary operand packs the batch of two consecutive steps side by side
(M = 128 = 64 batch x 2 steps), so the x-projection runs at full PE
utilization -- half the streamed columns of the per-step form. Results are
staged through an internal DRAM buffer (bf16) and DMA'd back one step
ahead of use. The per-step recurrence then only runs gh = h @ U (24
matmuls) plus gates; gx joins via DVE adds from SBUF.

The stacked-gx matmuls double as PE filler: they are emitted between one
step's recurrent matmuls and the next step's transposes, exactly covering
the previous step's ACT/DVE gate-chain latency that would otherwise stall
the PE (the transposes depend on h(i-1)).

All matmul operands are bf16 (fp32 accumulation); numpy-simulated end-to-end
relative error vs the fp32 reference is ~9e-3 against the 2e-2 budget.
z columns are negated on host so one sigmoid yields zneg = 1-z directly:
    h_new = h + zneg * (cand - h)
with h_cand = tanh(gx_h + r * hh) requiring hh separate from gx_h.

PSUM budget: ps {zneg|r|hh} [64,1536] x2 bufs = 6 banks; one shared
[128,512] pool (2 bufs) serves both the gx accumulators and the transpose
target pt = 8 banks total.

LEAD = 4 steps (2 pairs): gx for pair m+2 is computed during steps
(2m, 2m+1) -- 3 of 6 512-column blocks per step; x2/gxd are padded by 2
pairs so the tail's lookahead stays in bounds.
"""

import os
import numpy as np
from contextlib import ExitStack

# the staged gx buffer (404MB bf16) needs a larger DRAM scratchpad page
os.environ.setdefault("NEURON_SCRATCHPAD_PAGE_SIZE", "512")

B, T, D, H = 64, 1024, 512, 1024
KC = (D + H) // 128  # 12 K-chunks of the fused contraction layout (wu rows)
NCORES = 8
LEADP = 2  # pairs of lookahead

_cache = {}


def _build(n_steps, bf16=True):
    import concourse.bass as bass
    import concourse.tile as tile
    from concourse import bacc, mybir

    f32 = mybir.dt.float32
    mdt = mybir.dt.bfloat16
    AF = mybir.ActivationFunctionType
    npairs = n_steps // 2

    nc = bacc.Bacc(
        "TRN2", target_bir_lowering=False, debug=False, num_devices=NCORES
    )
    x2_d = nc.dram_tensor("x2", [(npairs + LEADP) * 128, 512], mdt, kind="ExternalInput").ap()
    wu_d = nc.dram_tensor("wu", [KC * 128, 3072], mdt, kind="ExternalInput").ap()
    id_d = nc.dram_tensor("ident", [128, 64], f32, kind="ExternalInput").ap()
    out_d = nc.dram_tensor("out", [64, 1024], f32, kind="ExternalOutput").ap()
    gx_d = nc.dram_tensor("gxd", [(npairs + LEADP) * 128, 3072], mdt, kind="Internal").ap()

    with tile.TileContext(nc) as tc, ExitStack() as ctx:
        const = ctx.enter_context(tc.tile_pool(name="const", bufs=1))
        state = ctx.enter_context(tc.tile_pool(name="state", bufs=1))
        xpool = ctx.enter_context(tc.tile_pool(name="xin", bufs=2))
        gipool = ctx.enter_context(tc.tile_pool(name="gxi", bufs=2))
        gopool = ctx.enter_context(tc.tile_pool(name="gxo", bufs=2))
        gates = ctx.enter_context(tc.tile_pool(name="gates", bufs=3))
        ppool = ctx.enter_context(tc.tile_pool(name="psum", bufs=2, space="PSUM"))
        tpool = ctx.enter_context(tc.tile_pool(name="psumT", bufs=2, space="PSUM"))

        # --- persistent SBUF ---
        wu_s = const.tile([128, KC * 3072], mdt, tag="wu")
        for c in range(KC):
            nc.sync.dma_start(
                wu_s[:, c * 3072 : (c + 1) * 3072],
                wu_d[c * 128 : (c + 1) * 128, :],
            )
        ident = const.tile([128, 64], f32, tag="ident")
        nc.sync.dma_start(ident[:], id_d[:])

        h_cur = [state.tile([64, 1024], f32, tag=f"hcur{p}", name=f"hcur{p}") for p in range(2)]
        hT = [state.tile([128, 512], mdt, tag=f"hT{p}", name=f"hT{p}") for p in range(2)]

        nc.vector.memset(h_cur[0][:], 0.0)

        def gx_blocks(x2_t, row_off, nlo, nhi):
            """Stacked x-projection for blocks [nlo, nhi) of the pair whose
            gxd rows start at row_off (AP row expression)."""
            for n in range(nlo, nhi):
                gps = tpool.tile([128, 512], f32, tag="pt")
                for c in range(4):
                    nc.tensor.matmul(
                        gps[:], x2_t[:, c * 128 : (c + 1) * 128],
                        wu_s[:, c * 3072 + n * 512 : c * 3072 + n * 512 + 512],
                        start=(c == 0), stop=(c == 3), skip_group_check=True,
                    )
                go = gopool.tile([128, 512], mdt, tag="go")
                nc.scalar.copy(go[:], gps[:])
                nc.sync.dma_start(gx_d[row_off, n * 512 : (n + 1) * 512], go[:])

        def h_mms(hT_t, ps, hf):
            for j in range(8):
                lhsT = hT_t[:, j * 64 : (j + 1) * 64]
                cb = (4 + j) * 3072 + hf * 512
                nc.tensor.matmul(
                    ps[:, 0:512], lhsT, wu_s[:, cb : cb + 512],
                    start=(j == 0), stop=(j == 7), skip_group_check=True,
                )
                nc.tensor.matmul(
                    ps[:, 512:1024], lhsT, wu_s[:, cb + 1024 : cb + 1536],
                    start=(j == 0), stop=(j == 7), skip_group_check=True,
                )
                nc.tensor.matmul(
                    ps[:, 1024:1536], lhsT, wu_s[:, cb + 2048 : cb + 2560],
                    start=(j == 0), stop=(j == 7), skip_group_check=True,
                )

        def gate_chain(ps, gxi, hf, h_in_s, h_out_s):
            """ACT/DVE gate math for one half ([64, 512] tensors)."""
            tr = gates.tile([64, 512], f32, tag="tr")
            nc.vector.tensor_add(tr[:], ps[:, 512:1024],
                                 gxi[:, 1024 + hf * 512 : 1536 + hf * 512])
            zrs = gates.tile([64, 1024], f32, tag="zrs")
            nc.scalar.activation(zrs[:, 512:1024], tr[:], AF.Sigmoid)
            t1 = gates.tile([64, 512], f32, tag="t1")
            nc.vector.tensor_mul(t1[:], zrs[:, 512:1024], ps[:, 1024:1536])
            tz = gates.tile([64, 512], f32, tag="tz")
            nc.vector.tensor_add(tz[:], ps[:, 0:512],
                                 gxi[:, hf * 512 : hf * 512 + 512])
            nc.scalar.activation(zrs[:, 0:512], tz[:], AF.Sigmoid)
            t2 = gates.tile([64, 512], f32, tag="t2")
            nc.vector.tensor_add(t2[:], t1[:],
                                 gxi[:, 2048 + hf * 512 : 2560 + hf * 512])
            cand = gates.tile([64, 512], f32, tag="cand")
            nc.scalar.activation(cand[:], t2[:], AF.Tanh)
            d = gates.tile([64, 512], f32, tag="d")
            nc.vector.tensor_sub(d[:], cand[:], h_in_s)
            e = gates.tile([64, 512], f32, tag="e")
            nc.vector.tensor_mul(e[:], zrs[:, 0:512], d[:])
            nc.vector.tensor_add(h_out_s, h_in_s, e[:])

        def step(iv, k, p, x2_t):
            """One GRU step (step index = iv + k, parity p)."""
            h_in, h_out = h_cur[p], h_cur[1 - p]

            # gx for this step (written LEADP pairs ago)
            gxi = gipool.tile([64, 3072], mdt, tag="gxi")
            nc.sync.dma_start(gxi[:], gx_d[bass.ds(iv * 64 + 64 * k, 64), :])

            # filler: stacked x-projection for pair (step + 2*LEADP)/2
            pair_row = bass.ds(iv * 64 + 64 * (k - p) + 128 * LEADP, 128)
            nlo = 3 * p
            gx_blocks(x2_t, pair_row, nlo, nlo + 3)

            # transpose h_in -> hT[p]
            pt = tpool.tile([128, 512], f32, tag="pt")
            for kk in range(8):
                nc.tensor.transpose(
                    pt[:, kk * 64 : (kk + 1) * 64],
                    h_in[:, kk * 128 : (kk + 1) * 128],
                    ident[0:64, :],
                )
            nc.vector.tensor_copy(hT[p][:, 0:256], pt[:, 0:256])
            nc.vector.tensor_copy(hT[p][:, 256:512], pt[:, 256:512])

            ps0 = ppool.tile([64, 1536], f32, tag="ps")
            h_mms(hT[p], ps0, 0)
            gate_chain(ps0, gxi, 0, h_in[:, 0:512], h_out[:, 0:512])

            ps1 = ppool.tile([64, 1536], f32, tag="ps")
            h_mms(hT[p], ps1, 1)
            gate_chain(ps1, gxi, 1, h_in[:, 512:1024], h_out[:, 512:1024])

        # --- prologue: gx for pairs 0..LEADP-1 ---
        for P in range(LEADP):
            x2_t = xpool.tile([128, 512], mdt, tag="x2")
            nc.sync.dma_start(x2_t[:], x2_d[P * 128 : (P + 1) * 128, :])
            gx_blocks(x2_t, slice(P * 128, (P + 1) * 128), 0, 6)

        with tc.For_i(0, n_steps, 4, hint_engines=(mybir.EngineType.PE,), staggered_reset=True) as i:
            x2_a = xpool.tile([128, 512], mdt, tag="x2")
            nc.sync.dma_start(x2_a[:], x2_d[bass.ds(i * 64 + 128 * LEADP, 128), :])
            step(i, 0, 0, x2_a)
            step(i, 1, 1, x2_a)
            x2_b = xpool.tile([128, 512], mdt, tag="x2")
            nc.sync.dma_start(x2_b[:], x2_d[bass.ds(i * 64 + 128 + 128 * LEADP, 128), :])
            step(i, 2, 0, x2_b)
            step(i, 3, 1, x2_b)

        nc.sync.dma_start(out_d[:], h_cur[0][:])

    nc.compile()
    return nc


def _host_prep(x, W, U):
    """Build x2 / wu host-side arrays for one head (bf16)."""
    import ml_dtypes
    n_steps = x.shape[1]
    npairs = n_steps // 2
    x2 = (
        x.transpose(1, 2, 0)                      # [T, D, B]
        .reshape(npairs, 2, 4, 128, B)            # [P, s, c, k, b]
        .transpose(0, 3, 2, 1, 4)                 # [P, k, c, s, b]
        .reshape(npairs * 128, 512)
        .astype(np.float32)
    )
    x2 = np.concatenate(
        [x2, np.zeros((LEADP * 128, 512), np.float32)], axis=0
    )
    Wp = np.asarray(W, np.float32)
    Up = np.asarray(U, np.float32)
    wu = np.concatenate([Wp, Up], axis=0).copy()  # [1536, 3072]
    wu[:, 0:H] *= -1.0  # negate z columns
    return (
        np.ascontiguousarray(x2.astype(ml_dtypes.bfloat16)),
        np.ascontiguousarray(wu.astype(ml_dtypes.bfloat16)),
    )


def _unpermute_h(res):
    return np.asarray(res, np.float32)


def _make_ident():
    id2 = np.zeros((128, 64), np.float32)
    for p in range(128):
        id2[p, p % 64] = 1.0
    return id2


def _run_spmd(nc, in_maps, n_timed=0):
    """Execute on the 8 axon cores via PJRT shard_map; keeps the jitted
    callable + device inputs resident so timed runs measure execution."""
    import time
    import jax
    from jax.sharding import Mesh, PartitionSpec
    from jax.experimental.shard_map import shard_map
    from concourse import bass2jax, mybir

    bass2jax.install_neuronx_cc_hook()
    n_cores = len(in_maps)

    in_names, out_names, out_avals = [], [], []
    partition_name = nc.partition_id_tensor.name if nc.partition_id_tensor else None
    for alloc in nc.m.functions[0].allocations:
        if not isinstance(alloc, mybir.MemoryLocationSet):
            continue
        name = alloc.memorylocations[0].name
        if alloc.kind == "ExternalInput":
            if name != partition_name:
                in_names.append(name)
        elif alloc.kind == "ExternalOutput":
            shape = tuple(alloc.tensor_shape)
            dtype = mybir.dt.np(alloc.dtype)
            out_avals.append(jax.core.ShapedArray(shape, dtype))
            out_names.append(name)
    n_params = len(in_names)
    n_outs = len(out_names)
    all_in = in_names + out_names
    if partition_name is not None:
        all_in.append(partition_name)

    def _body(*args):
        operands = list(args)
        if partition_name is not None:
            operands.append(bass2jax.partition_id_tensor())
        outs = bass2jax._bass_exec_p.bind(
            *operands,
            out_avals=tuple(out_avals),
            in_names=tuple(all_in),
            out_names=tuple(out_names),
            lowering_input_output_aliases=(),
            sim_require_finite=True,
            sim_require_nnan=True,
            nc=nc,
        )
        return tuple(outs)

    devices = jax.devices()[:n_cores]
    mesh = Mesh(np.asarray(devices), ("core",))
    in_specs = (PartitionSpec("core"),) * (n_params + n_outs)
    out_specs = (PartitionSpec("core"),) * n_outs
    sharded = jax.jit(
        shard_map(_body, mesh=mesh, in_specs=in_specs, out_specs=out_specs,
                  check_rep=False),
        keep_unused=True,
    )
    sharding = jax.sharding.NamedSharding(mesh, PartitionSpec("core"))

    def _stage(per_core_arrays):
        shards = []
        for c, arr in enumerate(per_core_arrays):
            sh = jax.device_put(np.asarray(arr), devices[c])
            sh.block_until_ready()
            shards.append(sh)
        a0 = np.asarray(per_core_arrays[0])
        gshape = (n_cores * a0.shape[0], *a0.shape[1:])
        return jax.make_array_from_single_device_arrays(gshape, sharding, shards)

    dev_in = [_stage([in_maps[c][nm] for c in range(n_cores)]) for nm in in_names]
    dev_zero = [
        _stage([np.zeros(av.shape, av.dtype) for _ in range(n_cores)])
        for av in out_avals
    ]
    for a in dev_in + dev_zero:
        a.block_until_ready()

    out_arrs = sharded(*dev_in, *dev_zero)
    jax.block_until_ready(out_arrs)

    best = None
    for _ in range(n_timed):
        t0 = time.perf_counter_ns()
        out_arrs = sharded(*dev_in, *dev_zero)
        jax.block_until_ready(out_arrs)
        dt = time.perf_counter_ns() - t0
        best = dt if best is None else min(best, dt)

    results = [
        {
            nm: np.asarray(out_arrs[i]).reshape(n_cores, *out_avals[i].shape)[c]
            for i, nm in enumerate(out_names)
        }
        for c in range(n_cores)
    ]
    return results, best


def kernel(x, W0, U0, bi0, br0, W1, U1, bi1, br1):
    x = np.asarray(x, dtype=np.float32)
    assert all(
        not np.any(np.asarray(b)) for b in (bi0, br0, bi1, br1)
    ), "nonzero biases not supported by this kernel build"

    n_steps = x.shape[1]
    key = n_steps
    if key not in _cache:
        _cache[key] = _build(n_steps)
    nc = _cache[key]

    x2, wu0 = _host_prep(x, np.asarray(W0), np.asarray(U0))
    _, wu1 = _host_prep(x[:, :2], np.asarray(W1), np.asarray(U1))
    ident = _make_ident()

    maps = []
    for core in range(NCORES):
        wu = wu0 if core % 2 == 0 else wu1
        maps.append({"x2": x2, "wu": wu, "ident": ident})

    n_timed = int(os.environ.get("GRU_TIMED_RUNS", "0"))
    results, best_ns = _run_spmd(nc, maps, n_timed=n_timed)
    kernel.last_exec_ns = best_ns
    out0 = _unpermute_h(results[0]["out"])
    out1 = _unpermute_h(results[1]["out"])
    return out0, out1


kernel.last_exec_ns = None
